# revision 14
# baseline (speedup 1.0000x reference)
"""Trainium2 Bass kernel for nn_Attention_16028817948779.

Reference computation (b=4, c=256, heads=8, d=64, h=w=48, n=2304):
  qkv = w_qkv @ x          (1x1 conv)
  q,k,v -> [b, H, d, n];  q,k l2-normalized along n (spatial)
  sim  = (q^T k) * 10;  attn = softmax(sim, axis=-1)
  out  = attn @ v^T -> [b, H, n, d] -> [b, H*d, h, w]
  y    = w_out @ out + b_out

Sharding: 8 cores; core c handles batch c//2, head group (c%2)*4..+4.
Each core computes a partial y over its 4 heads; host sums the two
partials per batch and adds the bias.

Kernel design (v2, bf16):
  - All matmul operands in bf16 (1 PE cycle/row vs ~1.5-2 for fp32r as
    measured on HW); inputs are cast to bf16 on the host so the DMA
    volume halves too. PSUM accumulation stays fp32; overall rel err
    ~5e-3 vs the 2e-2 gate.
  - Attention in transposed form ST[j,i] = k_j . q_i, so softmax needs no
    on-chip transposes: exp without max-subtraction is safe because q,k
    are l2-normalized along n (|sim| <= 10) and the softmax denominator
    comes for free from a ones-column appended to V^T in the PV matmul.
  - Two heads' ST matmuls run concurrently on the PE via row tiling
    (K=64 each; the head pair lives at partitions 0-63 / 64-127 of the
    QKV projection output).
  - 10/(|q_row| |k_row|) folded into a single per-row scale of q.
  - Softmax denominator: reciprocal_approx_fast straight from PSUM row
    64, partition-broadcast on the (otherwise idle) GpSimd engine, and
    one DVE multiply straight from PSUM -> bf16 outT. No DRAM roundtrip.
"""

import os
import sys

import numpy as np

_TRN_REPO = "/opt/trn_rl_repo"
if _TRN_REPO not in sys.path:
    sys.path.insert(0, _TRN_REPO)

B = 4
C = 256
HEADS = 8
D = 64
N = 2304  # 48*48
HID = HEADS * D  # 512

N_CORES = 8
CI = 2  # c chunks of 128
# i/n chunks of <=512 (PSUM bank / fp32 moving-operand limit)
NCHUNKS = [(0, 512), (512, 512), (1024, 512), (1536, 512), (2048, 256)]
NJ = N // 128  # 18 key chunks of 128


def _apply_compat_patches():
    """walrus in this env only accepts ~1 sync wait per instruction, but the
    Tile framework attaches one wait per outstanding proc to a single
    instruction. Split excess waits onto EventSemaphore instructions at the
    BIR-JSON level (Bass.to_json_bytes is the serialization choke point for
    both the native and the axon/PJRT compile paths)."""
    import json

    import concourse.bass as bass

    if getattr(bass.Bass.to_json_bytes, "_waitsplit", False):
        return

    MAXW = 1
    _orig = bass.Bass.to_json_bytes

    def _split_waits(raw):
        m = json.loads(raw)
        ctr = 0
        changed = False
        for f in m.get("functions", []):
            for blk in f.get("blocks", []):
                new_insts = []
                for ins in blk.get("instructions", []):
                    si = ins.get("sync_info")
                    waits = (si or {}).get("on_wait") or []
                    if len(waits) > MAXW:
                        changed = True
                        for w in waits[:-MAXW]:
                            ctr += 1
                            new_insts.append(
                                {
                                    "debug": ins.get("debug", 0),
                                    "engine": ins["engine"],
                                    "ins": [],
                                    "outs": [],
                                    "name": f"waitsplit_{ctr}",
                                    "opcode": "EventSemaphore",
                                    "sync_info": {"on_update": [], "on_wait": [w]},
                                }
                            )
                        si["on_wait"] = waits[-MAXW:]
                    new_insts.append(ins)
                blk["instructions"] = new_insts
        return json.dumps(m).encode() if changed else raw

    def _patched(self):
        return _split_waits(_orig(self))

    _patched._waitsplit = True
    bass.Bass.to_json_bytes = _patched


def build_kernel():
    import concourse.bass as bass
    import concourse.mybir as mybir
    import concourse.tile as tile

    _apply_compat_patches()

    f32 = mybir.dt.float32
    bf16 = mybir.dt.bfloat16
    i16 = mybir.dt.int16
    Exp = mybir.ActivationFunctionType.Exp
    Ln = mybir.ActivationFunctionType.Ln
    Square = mybir.ActivationFunctionType.Square
    mult = mybir.AluOpType.mult
    add = mybir.AluOpType.add
    X = mybir.AxisListType.X

    # Schraudolph bf16 exp: bits(int16(A*x + B)) read as bf16 ~= e^x
    # (max rel err 3.3%, verified bit-exact vs HW: DVE rounds-to-nearest).
    # Every third j-block's softmax exp runs on the DVE this way, relieving
    # the ACT engine (the kernel's bottleneck); softmax tolerates the error
    # (end-to-end rel err ~1.4e-2 vs the 2e-2 gate).
    SCH_A = float(128 * np.log2(np.e))
    SCH_B = float(127 * 128 - 5.5)

    nc = bass.Bass()
    x_d = nc.dram_tensor("x", [C, N], bf16, kind="ExternalInput")
    wqT_d = nc.dram_tensor("wqT", [C, 256], bf16, kind="ExternalInput")
    wkT_d = nc.dram_tensor("wkT", [C, 256], bf16, kind="ExternalInput")
    wvT_d = nc.dram_tensor("wvT", [C, 256], bf16, kind="ExternalInput")
    woutT_d = nc.dram_tensor("woutT", [64, 4, 256], bf16, kind="ExternalInput")
    y_d = nc.dram_tensor("y", [C, N], f32, kind="ExternalOutput")

    with tile.TileContext(nc) as tc:
        with (
            tc.tile_pool(name="persist", bufs=1) as pp,
            tc.tile_pool(name="pt", bufs=4) as ptp,
            tc.tile_pool(name="misc", bufs=2) as mp,
            tc.tile_pool(name="dram", bufs=2, space="DRAM") as dp,
            tc.tile_pool(name="ps_st", bufs=2, space="PSUM") as ps_st,
            tc.tile_pool(name="ps_pv", bufs=4, space="PSUM") as ps_pv,
        ):
            # ---- load inputs ----
            x_sb = pp.tile([128, CI, N], bf16)
            for ci in range(CI):
                for ns, nl in NCHUNKS:
                    nc.sync.dma_start(
                        out=x_sb[:, ci, ns : ns + nl],
                        in_=x_d[ci * 128 : (ci + 1) * 128, ns : ns + nl],
                    )
            wq_sb = pp.tile([128, CI, 256], bf16)
            wk_sb = pp.tile([128, CI, 256], bf16)
            wv_sb = pp.tile([128, CI, 256], bf16)
            for w_sb, w_d in ((wq_sb, wqT_d), (wk_sb, wkT_d), (wv_sb, wvT_d)):
                nc.sync.dma_start(
                    out=w_sb[:], in_=w_d.rearrange("(ci p) o -> p ci o", p=128)
                )
            wo_sb = pp.tile([64, 4, 256], bf16)
            nc.sync.dma_start(out=wo_sb[:], in_=woutT_d[:])

            ones_f = pp.tile([128, 1], bf16)
            nc.vector.memset(ones_f[:], 1.0)
            ones64 = pp.tile([1, 64], f32)
            nc.vector.memset(ones64[:], 1.0)

            # PE warm-up: dummy bf16 matmuls with no input dependencies,
            # executed during the initial DMA wait so the HAM clock gate is
            # already at 8/8 when the real QKV matmuls arrive.
            warm_sb = pp.tile([128, 512], bf16)
            nc.vector.memset(warm_sb[:], 1.0)
            warm_ps = ps_st.tile([128, 2, 512], f32, tag="st", name="warm_ps")
            NWARM = 12
            for wi in range(NWARM):
                nc.tensor.matmul(
                    warm_ps[:, 0, :],
                    lhsT=warm_sb[:, 0:128],
                    rhs=warm_sb[:],
                    start=(wi == 0),
                    stop=(wi == NWARM - 1),
                )
            nc.vector.tensor_copy(warm_sb[:, 0:16], warm_ps[:, 0, 0:16])

            # vt_sb: [n-part, j-chunk, 4*65]; per head 64 v columns + a ones col
            vt_sb = pp.tile([128, NJ, 260], bf16)
            vt4 = vt_sb.rearrange("p j (h e) -> p j h e", e=65)
            nc.vector.tensor_copy(
                vt4[:, :, :, 64:65],
                ones_f[:, 0:1].unsqueeze(1).unsqueeze(1).to_broadcast((128, NJ, 4, 1)),
            )
            for j in range(NJ):
                ps = ps_pv.tile([128, 256], f32, tag="pv", name="vt_ps")
                for ci in range(CI):
                    nc.tensor.matmul(
                        ps[:],
                        lhsT=x_sb[:, ci, j * 128 : (j + 1) * 128],
                        rhs=wv_sb[:, ci, :],
                        start=(ci == 0),
                        stop=(ci == CI - 1),
                    )
                eng = nc.scalar.copy if j % 2 == 0 else nc.vector.tensor_copy
                eng(vt4[:, j, :, 0:64], ps.rearrange("p (h d) -> p h d", h=4))

            # ---- QKV projection ----
            # q_sb/k_sb: [d-part, head-pair, n]; heads 2p at part 0-63, 2p+1 at 64-127
            q_sb = pp.tile([128, 2, N], bf16)
            k_sb = pp.tile([128, 2, N], bf16)
            # per-(tensor, oc) partial sum-of-squares, one column per n-chunk,
            # computed chunk-wise right behind the projection so the PE never
            # waits on the l2norm reduction later
            ssq = mp.tile([128, 2, 2, len(NCHUNKS)], f32, tag="ssq")
            scratch = pp.tile([128, 512], f32)
            for ti, (dst, w_sb) in enumerate(((q_sb, wq_sb), (k_sb, wk_sb))):
                for oc in range(2):
                    for nci, (ns, nl) in enumerate(NCHUNKS):
                        ps = ps_pv.tile([128, 512], f32, tag="pv", name="qk_ps")
                        for ci in range(CI):
                            nc.tensor.matmul(
                                ps[:, :nl],
                                lhsT=w_sb[:, ci, oc * 128 : (oc + 1) * 128],
                                rhs=x_sb[:, ci, ns : ns + nl],
                                start=(ci == 0),
                                stop=(ci == CI - 1),
                            )
                        nc.vector.tensor_copy(dst[:, oc, ns : ns + nl], ps[:, :nl])
                        nc.scalar.activation(
                            scratch[:, :nl],
                            ps[:, :nl],
                            Square,
                            accum_out=ssq[:, ti, oc, nci : nci + 1],
                        )

            # ---- fold l2norm + SCALE into q: q *= 10/sqrt(ssq_q*ssq_k) per row ----
            sqk = mp.tile([128, 2, 2], f32, tag="sqk")
            nc.vector.reduce_sum(
                sqk.rearrange("p a b -> p (a b)"),
                ssq.rearrange("p a b c -> p (a b) c"),
                axis=X,
            )
            qscale = mp.tile([128, 2], f32, tag="qscale")
            nc.vector.tensor_tensor(qscale[:], sqk[:, 0, :], sqk[:, 1, :], mult)
            # 10/sqrt(x) = exp(-0.5*ln(x) + ln(10)); Ln and Exp share one ACT
            # table set, so no extra table load next to the softmax exps
            nc.scalar.activation(qscale[:], qscale[:], Ln)
            ln10 = mp.tile([128, 1], f32, tag="ln10")
            nc.vector.memset(ln10[:], 2.302585092994046)
            nc.scalar.activation(
                qscale[:], qscale[:], Exp, bias=ln10[:], scale=-0.5
            )
            with nc.allow_low_precision(reason="q scale written as bf16"):
                # chunk-split so the first ST matmuls only wait on chunk 0
                for ns, nl in NCHUNKS:
                    for oc in range(2):
                        nc.vector.tensor_scalar_mul(
                            q_sb[:, oc, ns : ns + nl],
                            q_sb[:, oc, ns : ns + nl],
                            qscale[:, oc : oc + 1],
                        )

            # ---- attention per head pair p (local heads 2p, 2p+1) ----
            outT = [
                pp.tile([64, N], bf16, name=f"outT{h}", tag=f"outT{h}")
                for h in range(4)
            ]

            def emit_proj(ns, il):
                for oc_ in range(2):
                    yps = ps_pv.tile([128, 512], f32, tag="pv", name="yps")
                    for h in range(4):
                        nc.tensor.matmul(
                            yps[:, :il],
                            lhsT=wo_sb[:, h, oc_ * 128 : (oc_ + 1) * 128],
                            rhs=outT[h][:, ns : ns + il],
                            start=(h == 0),
                            stop=(h == 3),
                        )
                    y_sb = mp.tile([128, 512], f32, tag="ysb", name="y_sb")
                    nc.vector.tensor_copy(y_sb[:, :il], yps[:, :il])
                    nc.sync.dma_start(
                        out=y_d[oc_ * 128 : (oc_ + 1) * 128, ns : ns + il],
                        in_=y_sb[:, :il],
                    )

            for p in range(2):
                for nci, (ns, il) in enumerate(NCHUNKS):
                    hA, hB = 2 * p, 2 * p + 1
                    pvA = ps_pv.tile([65, 512], f32, tag="pv", name="pvA")
                    pvB = ps_pv.tile([65, 512], f32, tag="pv", name="pvB")

                    def emit_pv(jj, pt):
                        nc.tensor.matmul(
                            pvA[:, :il],
                            lhsT=vt4[:, jj, hA, :],
                            rhs=pt[:, 0, :il],
                            start=(jj == 0),
                            stop=(jj == NJ - 1),
                        )
                        nc.tensor.matmul(
                            pvB[:, :il],
                            lhsT=vt4[:, jj, hB, :],
                            rhs=pt[:, 1, :il],
                            start=(jj == 0),
                            stop=(jj == NJ - 1),
                        )

                    # Software-pipeline the PV matmuls two steps behind the
                    # ST/exp stream: the PE queue is strictly in-order, so
                    # with PV(j) emitted right after ST(j) the queue stalls
                    # on exp(j) (~1.2us latency) every step. With lag 2 the
                    # exp hides behind two steps of PE work.
                    pts = {}
                    for j in range(NJ):
                        st = ps_st.tile([128, 2, 512], f32, tag="st", name="st")
                        nc.tensor.matmul(
                            st[:, 0, :il],
                            lhsT=k_sb[0:64, p, j * 128 : (j + 1) * 128],
                            rhs=q_sb[0:64, p, ns : ns + il],
                        )
                        nc.tensor.matmul(
                            st[:, 1, :il],
                            lhsT=k_sb[64:128, p, j * 128 : (j + 1) * 128],
                            rhs=q_sb[64:128, p, ns : ns + il],
                        )
                        pt = ptp.tile([128, 2, 512], bf16, tag="pt", name="pt")
                        if j % 3 == 2:
                            # Schraudolph exp on the DVE (ACT is the
                            # bottleneck engine; this takes every 3rd j)
                            with nc.allow_low_precision(
                                reason="approx exp bits written as int16"
                            ):
                                nc.vector.tensor_scalar(
                                    out=pt.bitcast(i16)[:, :, :il],
                                    in0=st[:, :, :il],
                                    scalar1=SCH_A,
                                    scalar2=SCH_B,
                                    op0=mult,
                                    op1=add,
                                )
                        else:
                            nc.scalar.activation(pt[:, :, :il], st[:, :, :il], Exp)
                        emit_pv(j, pt)

                    # Project the i-block from TWO chunks ago: its normalize
                    # chain (DMA roundtrips) finished long ago, so even if the
                    # Tile scheduler hoists these matmuls ahead of this
                    # chunk's PV stream, they never block the PE queue.
                    if p == 1 and nci >= 2:
                        emit_proj(*NCHUNKS[nci - 2])

                    # Normalize rows 0-63 by row 64 (softmax denominator).
                    # Copy both PV accumulators to SBUF immediately (frees the
                    # PSUM slots), then: both heads' denominator rows ->
                    # DRAM -> back as [128, W/128] so the exact reciprocal
                    # runs 128-wide on the DVE (a [1,512] reciprocal costs
                    # 3.6us; this costs ~0.1us) -> DRAM -> partition-broadcast
                    # read; final multiply on the otherwise idle GpSimd.
                    nsbs = []
                    for pv, h in ((pvA, hA), (pvB, hB)):
                        nsb = mp.tile([65, 512], f32, tag="nsb", name="nsb", bufs=4)
                        nc.vector.tensor_copy(nsb[:, :il], pv[:, :il])
                        nsbs.append((nsb, h))

                    if p == 1 and nci == len(NCHUNKS) - 1:
                        # Tail short-path: the final chunk's normalize gates
                        # the last projection, so skip the DMA roundtrips:
                        # exact DVE reciprocal of the (short) denominator row,
                        # then broadcast across partitions with a K=1 ones
                        # matmul on the PE, multiply straight from PSUM.
                        with nc.allow_low_precision(
                            reason="attn out written bf16"
                        ):
                            for nsb, h in nsbs:
                                rden = mp.tile(
                                    [1, 512], f32, tag="rden", name="rden", bufs=2
                                )
                                nc.vector.reciprocal(
                                    rden[:, :il], nsb[64:65, :il]
                                )
                                bc_ps = ps_st.tile(
                                    [128, 2, 512], f32, tag="st", name="bc_ps"
                                )
                                nc.tensor.matmul(
                                    bc_ps[0:64, 0, :il],
                                    lhsT=ones64[:],
                                    rhs=rden[:, :il],
                                )
                                nc.vector.tensor_tensor(
                                    outT[h][:, ns : ns + il],
                                    nsb[0:64, :il],
                                    bc_ps[0:64, 0, :il],
                                    mult,
                                )
                        continue

                    W = 2 * il
                    rd = dp.tile([1, 1024], f32, tag="rd", name="rd")
                    rr = dp.tile([1, 1024], f32, tag="rr", name="rr")
                    for idx, (nsb, h) in enumerate(nsbs):
                        nc.sync.dma_start(
                            out=rd[:, idx * il : (idx + 1) * il],
                            in_=nsb[64:65, :il],
                        )
                    at = mp.tile([128, 8], f32, tag="at", name="at", bufs=4)
                    rc = mp.tile([128, 8], f32, tag="rc", name="rc", bufs=4)
                    nc.sync.dma_start(
                        out=at[:, : W // 128],
                        in_=rd[:, :W].rearrange("o (p c) -> (o p) c", p=128),
                    )
                    nc.vector.reciprocal(rc[:, : W // 128], at[:, : W // 128])
                    nc.sync.dma_start(
                        out=rr[:, :W].rearrange("o (p c) -> (o p) c", p=128),
                        in_=rc[:, : W // 128],
                    )
                    with nc.allow_low_precision(reason="attn out written bf16"):
                        for idx, (nsb, h) in enumerate(nsbs):
                            bc_sb = mp.tile(
                                [64, 512], f32, tag="bcsb", name="bc_sb", bufs=4
                            )
                            nc.sync.dma_start(
                                out=bc_sb[:, :il],
                                in_=rr[0:1, idx * il : idx * il + il].to_broadcast(
                                    (64, il)
                                ),
                            )
                            nc.gpsimd.tensor_tensor(
                                outT[h][:, ns : ns + il],
                                nsb[0:64, :il],
                                bc_sb[:, :il],
                                mult,
                            )

            emit_proj(*NCHUNKS[-2])
            emit_proj(*NCHUNKS[-1])

    return nc


_NC_CACHE = None


def kernel(x, w_qkv, w_out, b_out):
    global _NC_CACHE
    import ml_dtypes
    from concourse.bass_utils import run_bass_kernel_spmd

    bf = ml_dtypes.bfloat16
    x = np.ascontiguousarray(x, dtype=np.float32)
    w_qkv = np.asarray(w_qkv, dtype=np.float32)
    w_out = np.asarray(w_out, dtype=np.float32)
    b_out = np.asarray(b_out, dtype=np.float32)

    b, c, h, w = x.shape
    assert (b, c, h, w) == (B, C, 48, 48)
    x_bn = x.reshape(B, C, N)

    wq, wk, wv = w_qkv[0:HID], w_qkv[HID : 2 * HID], w_qkv[2 * HID : 3 * HID]
    w_outT = np.ascontiguousarray(w_out.T)  # [HID, C]

    in_maps = []
    for core in range(N_CORES):
        bb, g = core // 2, core % 2
        rows = slice(g * 256, g * 256 + 256)
        woutT_c = np.ascontiguousarray(
            w_outT[rows].reshape(4, 64, 256).transpose(1, 0, 2).astype(bf)
        )
        in_maps.append(
            {
                "x": np.ascontiguousarray(x_bn[bb].astype(bf)),
                "wqT": np.ascontiguousarray(wq[rows].T.astype(bf)),
                "wkT": np.ascontiguousarray(wk[rows].T.astype(bf)),
                "wvT": np.ascontiguousarray(wv[rows].T.astype(bf)),
                "woutT": woutT_c,
            }
        )

    if _NC_CACHE is None:
        _NC_CACHE = build_kernel()
    nc = _NC_CACHE

    trace = bool(int(os.environ.get("KERNEL_TRACE", "0")))
    res = run_bass_kernel_spmd(
        nc,
        in_maps,
        core_ids=list(range(N_CORES)),
        trace=trace,
        trace_cores=list(range(N_CORES)) if trace else None,
    )
    kernel.last_result = res

    y = np.empty((B, C, N), dtype=np.float32)
    for bb in range(B):
        y[bb] = (
            res.results[2 * bb]["y"]
            + res.results[2 * bb + 1]["y"]
            + b_out[:, None]
        )
    return y.reshape(B, C, 48, 48)


# revision 16
# speedup vs baseline: 1.3723x; 1.3723x over previous
"""Trainium2 Bass kernel for nn_Attention_16028817948779.

Reference computation (b=4, c=256, heads=8, d=64, h=w=48, n=2304):
  qkv = w_qkv @ x          (1x1 conv)
  q,k,v -> [b, H, d, n];  q,k l2-normalized along n (spatial)
  sim  = (q^T k) * 10;  attn = softmax(sim, axis=-1)
  out  = attn @ v^T -> [b, H, n, d] -> [b, H*d, h, w]
  y    = w_out @ out + b_out

Sharding: 8 cores; core c handles batch c//2, head group (c%2)*4..+4.
Each core computes a partial y over its 4 heads; host sums the two
partials per batch and adds the bias.

Kernel design (v2, bf16):
  - All matmul operands in bf16 (1 PE cycle/row vs ~1.5-2 for fp32r as
    measured on HW); inputs are cast to bf16 on the host so the DMA
    volume halves too. PSUM accumulation stays fp32; overall rel err
    ~5e-3 vs the 2e-2 gate.
  - Attention in transposed form ST[j,i] = k_j . q_i, so softmax needs no
    on-chip transposes: exp without max-subtraction is safe because q,k
    are l2-normalized along n (|sim| <= 10) and the softmax denominator
    comes for free from a ones-column appended to V^T in the PV matmul.
  - Two heads' ST matmuls run concurrently on the PE via row tiling
    (K=64 each; the head pair lives at partitions 0-63 / 64-127 of the
    QKV projection output).
  - 10/(|q_row| |k_row|) folded into a single per-row scale of q.
  - Softmax denominator: reciprocal_approx_fast straight from PSUM row
    64, partition-broadcast on the (otherwise idle) GpSimd engine, and
    one DVE multiply straight from PSUM -> bf16 outT. No DRAM roundtrip.
"""

import os
import sys

import numpy as np

_TRN_REPO = "/opt/trn_rl_repo"
if _TRN_REPO not in sys.path:
    sys.path.insert(0, _TRN_REPO)

B = 4
C = 256
HEADS = 8
D = 64
N = 2304  # 48*48
HID = HEADS * D  # 512

N_CORES = 8
CI = 2  # c chunks of 128
# i/n chunks of <=512 (PSUM bank / fp32 moving-operand limit)
NCHUNKS = [(0, 512), (512, 512), (1024, 512), (1536, 512), (2048, 256)]
NJ = N // 128  # 18 key chunks of 128


def _apply_compat_patches():
    """walrus in this env only accepts ~1 sync wait per instruction, but the
    Tile framework attaches one wait per outstanding proc to a single
    instruction. Split excess waits onto EventSemaphore instructions at the
    BIR-JSON level (Bass.to_json_bytes is the serialization choke point for
    both the native and the axon/PJRT compile paths)."""
    import json

    import concourse.bass as bass

    if getattr(bass.Bass.to_json_bytes, "_waitsplit", False):
        return

    MAXW = 1
    _orig = bass.Bass.to_json_bytes

    def _split_waits(raw):
        m = json.loads(raw)
        ctr = 0
        changed = False
        for f in m.get("functions", []):
            for blk in f.get("blocks", []):
                new_insts = []
                for ins in blk.get("instructions", []):
                    si = ins.get("sync_info")
                    waits = (si or {}).get("on_wait") or []
                    if len(waits) > MAXW:
                        changed = True
                        for w in waits[:-MAXW]:
                            ctr += 1
                            new_insts.append(
                                {
                                    "debug": ins.get("debug", 0),
                                    "engine": ins["engine"],
                                    "ins": [],
                                    "outs": [],
                                    "name": f"waitsplit_{ctr}",
                                    "opcode": "EventSemaphore",
                                    "sync_info": {"on_update": [], "on_wait": [w]},
                                }
                            )
                        si["on_wait"] = waits[-MAXW:]
                    new_insts.append(ins)
                blk["instructions"] = new_insts
        return json.dumps(m).encode() if changed else raw

    def _patched(self):
        return _split_waits(_orig(self))

    _patched._waitsplit = True
    bass.Bass.to_json_bytes = _patched


def build_kernel():
    import concourse.bass as bass
    import concourse.mybir as mybir
    import concourse.tile as tile

    _apply_compat_patches()

    f32 = mybir.dt.float32
    bf16 = mybir.dt.bfloat16
    i16 = mybir.dt.int16
    Exp = mybir.ActivationFunctionType.Exp
    Ln = mybir.ActivationFunctionType.Ln
    Square = mybir.ActivationFunctionType.Square
    mult = mybir.AluOpType.mult
    add = mybir.AluOpType.add
    X = mybir.AxisListType.X

    # Schraudolph bf16 exp: bits(int16(A*x + B)) read as bf16 ~= e^x
    # (max rel err 3.3%, verified bit-exact vs HW: DVE rounds-to-nearest).
    # Every third j-block's softmax exp runs on the DVE this way, relieving
    # the ACT engine (the kernel's bottleneck); softmax tolerates the error
    # (end-to-end rel err ~1.4e-2 vs the 2e-2 gate).
    SCH_A = float(128 * np.log2(np.e))
    SCH_B = float(127 * 128 - 5.5)

    nc = bass.Bass()
    x_d = nc.dram_tensor("x", [C, N], bf16, kind="ExternalInput")
    wqT_d = nc.dram_tensor("wqT", [C, 256], bf16, kind="ExternalInput")
    wkT_d = nc.dram_tensor("wkT", [C, 256], bf16, kind="ExternalInput")
    wvT_d = nc.dram_tensor("wvT", [C, 256], bf16, kind="ExternalInput")
    woutT_d = nc.dram_tensor("woutT", [128, 2, 256], bf16, kind="ExternalInput")
    y_d = nc.dram_tensor("y", [C, N], f32, kind="ExternalOutput")

    with tile.TileContext(nc) as tc:
        with (
            tc.tile_pool(name="persist", bufs=1) as pp,
            tc.tile_pool(name="pt", bufs=6) as ptp,
            tc.tile_pool(name="misc", bufs=2) as mp,
            tc.tile_pool(name="dram", bufs=2, space="DRAM") as dp,
            tc.tile_pool(name="ps_st", bufs=2, space="PSUM") as ps_st,
            tc.tile_pool(name="ps_pv", bufs=4, space="PSUM") as ps_pv,
        ):
            # ---- load inputs ----
            x_sb = pp.tile([128, CI, N], bf16)
            for ci in range(CI):
                for ns, nl in NCHUNKS:
                    nc.sync.dma_start(
                        out=x_sb[:, ci, ns : ns + nl],
                        in_=x_d[ci * 128 : (ci + 1) * 128, ns : ns + nl],
                    )
            wq_sb = pp.tile([128, CI, 256], bf16)
            wk_sb = pp.tile([128, CI, 256], bf16)
            wv_sb = pp.tile([128, CI, 256], bf16)
            for w_sb, w_d in ((wq_sb, wqT_d), (wk_sb, wkT_d), (wv_sb, wvT_d)):
                nc.sync.dma_start(
                    out=w_sb[:], in_=w_d.rearrange("(ci p) o -> p ci o", p=128)
                )
            wo_sb = pp.tile([128, 2, 256], bf16)
            nc.sync.dma_start(out=wo_sb[:], in_=woutT_d[:])

            ones_f = pp.tile([128, 1], bf16)
            nc.vector.memset(ones_f[:], 1.0)
            ones64 = pp.tile([1, 64], f32)
            nc.vector.memset(ones64[:], 1.0)

            # PE warm-up: dummy bf16 matmuls with no input dependencies,
            # executed during the initial DMA wait so the HAM clock gate is
            # already at 8/8 when the real QKV matmuls arrive.
            warm_sb = pp.tile([128, 512], bf16)
            nc.vector.memset(warm_sb[:], 1.0)
            warm_ps = ps_st.tile([128, 2, 512], f32, tag="st", name="warm_ps")
            NWARM = 8
            for wi in range(NWARM):
                nc.tensor.matmul(
                    warm_ps[:, 0, :],
                    lhsT=warm_sb[:, 0:128],
                    rhs=warm_sb[:],
                    start=(wi == 0),
                    stop=(wi == NWARM - 1),
                )
            nc.vector.tensor_copy(warm_sb[:, 0:16], warm_ps[:, 0, 0:16])

            # vt_sb: [n-part, j-chunk, 4*65]; per head 64 v columns + a ones col
            vt_sb = pp.tile([128, NJ, 260], bf16)
            vt4 = vt_sb.rearrange("p j (h e) -> p j h e", e=65)
            nc.vector.tensor_copy(
                vt4[:, :, :, 64:65],
                ones_f[:, 0:1].unsqueeze(1).unsqueeze(1).to_broadcast((128, NJ, 4, 1)),
            )
            for j in range(NJ):
                ps = ps_pv.tile([128, 256], f32, tag="pv", name="vt_ps")
                for ci in range(CI):
                    nc.tensor.matmul(
                        ps[:],
                        lhsT=x_sb[:, ci, j * 128 : (j + 1) * 128],
                        rhs=wv_sb[:, ci, :],
                        start=(ci == 0),
                        stop=(ci == CI - 1),
                    )
                eng = nc.scalar.copy if j % 2 == 0 else nc.vector.tensor_copy
                eng(vt4[:, j, :, 0:64], ps.rearrange("p (h d) -> p h d", h=4))

            # ---- QKV projection ----
            # q_sb/k_sb: [d-part, head-pair, n]; heads 2p at part 0-63, 2p+1 at 64-127
            q_sb = pp.tile([128, 2, N], bf16)
            k_sb = pp.tile([128, 2, N], bf16)
            # per-(tensor, oc) partial sum-of-squares, one column per n-chunk,
            # computed chunk-wise right behind the projection so the PE never
            # waits on the l2norm reduction later
            ssq = mp.tile([128, 2, 2, len(NCHUNKS)], f32, tag="ssq")
            scratch = pp.tile([128, 512], f32)
            for ti, (dst, w_sb) in enumerate(((q_sb, wq_sb), (k_sb, wk_sb))):
                for oc in range(2):
                    for nci, (ns, nl) in enumerate(NCHUNKS):
                        ps = ps_pv.tile([128, 512], f32, tag="pv", name="qk_ps")
                        for ci in range(CI):
                            nc.tensor.matmul(
                                ps[:, :nl],
                                lhsT=w_sb[:, ci, oc * 128 : (oc + 1) * 128],
                                rhs=x_sb[:, ci, ns : ns + nl],
                                start=(ci == 0),
                                stop=(ci == CI - 1),
                            )
                        nc.vector.tensor_copy(dst[:, oc, ns : ns + nl], ps[:, :nl])
                        nc.scalar.activation(
                            scratch[:, :nl],
                            ps[:, :nl],
                            Square,
                            accum_out=ssq[:, ti, oc, nci : nci + 1],
                        )

            # ---- fold l2norm + SCALE into q: q *= 10/sqrt(ssq_q*ssq_k) per row ----
            sqk = mp.tile([128, 2, 2], f32, tag="sqk")
            nc.vector.reduce_sum(
                sqk.rearrange("p a b -> p (a b)"),
                ssq.rearrange("p a b c -> p (a b) c"),
                axis=X,
            )
            qscale = mp.tile([128, 2], f32, tag="qscale")
            nc.vector.tensor_tensor(qscale[:], sqk[:, 0, :], sqk[:, 1, :], mult)
            # 10/sqrt(x) = exp(-0.5*ln(x) + ln(10)); Ln and Exp share one ACT
            # table set, so no extra table load next to the softmax exps
            nc.scalar.activation(qscale[:], qscale[:], Ln)
            ln10 = mp.tile([128, 1], f32, tag="ln10")
            nc.vector.memset(ln10[:], 2.302585092994046)
            nc.scalar.activation(
                qscale[:], qscale[:], Exp, bias=ln10[:], scale=-0.5
            )
            with nc.allow_low_precision(reason="q scale written as bf16"):
                # chunk-split so the first ST matmuls only wait on chunk 0
                for ns, nl in NCHUNKS:
                    for oc in range(2):
                        nc.vector.tensor_scalar_mul(
                            q_sb[:, oc, ns : ns + nl],
                            q_sb[:, oc, ns : ns + nl],
                            qscale[:, oc : oc + 1],
                        )

            # ---- attention per head pair p (local heads 2p, 2p+1) ----
            # outT[p] stacks the pair's two heads: head 2p at partitions
            # 0-63, head 2p+1 at 64-127 (engines support partition-shifted
            # writes), so the projection contracts K=128 per pair.
            outT = [
                pp.tile([128, N], bf16, name=f"outT{pp_}", tag=f"outT{pp_}")
                for pp_ in range(2)
            ]

            def emit_proj(ns, il):
                for oc_ in range(2):
                    yps = ps_pv.tile([128, 512], f32, tag="pv", name="yps")
                    for pr in range(2):
                        nc.tensor.matmul(
                            yps[:, :il],
                            lhsT=wo_sb[:, pr, oc_ * 128 : (oc_ + 1) * 128],
                            rhs=outT[pr][:, ns : ns + il],
                            start=(pr == 0),
                            stop=(pr == 1),
                        )
                    y_sb = mp.tile([128, 512], f32, tag="ysb", name="y_sb")
                    nc.vector.tensor_copy(y_sb[:, :il], yps[:, :il])
                    nc.sync.dma_start(
                        out=y_d[oc_ * 128 : (oc_ + 1) * 128, ns : ns + il],
                        in_=y_sb[:, :il],
                    )

            for p in range(2):
                for nci, (ns, il) in enumerate(NCHUNKS):
                    hA, hB = 2 * p, 2 * p + 1
                    pvA = ps_pv.tile([65, 512], f32, tag="pv", name="pvA")
                    pvB = ps_pv.tile([65, 512], f32, tag="pv", name="pvB")

                    def emit_pv(jj, pt):
                        nc.tensor.matmul(
                            pvA[:, :il],
                            lhsT=vt4[:, jj, hA, :],
                            rhs=pt[:, 0, :il],
                            start=(jj == 0),
                            stop=(jj == NJ - 1),
                        )
                        nc.tensor.matmul(
                            pvB[:, :il],
                            lhsT=vt4[:, jj, hB, :],
                            rhs=pt[:, 1, :il],
                            start=(jj == 0),
                            stop=(jj == NJ - 1),
                        )

                    # Software-pipeline the PV matmuls two steps behind the
                    # ST/exp stream: the PE queue is strictly in-order, so
                    # with PV(j) emitted right after ST(j) the queue stalls
                    # on exp(j) (~1.2us latency) every step. With lag 2 the
                    # exp hides behind two steps of PE work.
                    pts = {}
                    for j in range(NJ):
                        st = ps_st.tile([128, 2, 512], f32, tag="st", name="st")
                        nc.tensor.matmul(
                            st[:, 0, :il],
                            lhsT=k_sb[0:64, p, j * 128 : (j + 1) * 128],
                            rhs=q_sb[0:64, p, ns : ns + il],
                        )
                        nc.tensor.matmul(
                            st[:, 1, :il],
                            lhsT=k_sb[64:128, p, j * 128 : (j + 1) * 128],
                            rhs=q_sb[64:128, p, ns : ns + il],
                        )
                        pt = ptp.tile([128, 2, 512], bf16, tag="pt", name="pt")
                        if (j % 2 == 1) if il == 256 else (j % 4 == 3):
                            # Schraudolph exp on the DVE (ACT is the
                            # bottleneck engine; this takes every 3rd j)
                            with nc.allow_low_precision(
                                reason="approx exp bits written as int16"
                            ):
                                nc.vector.tensor_scalar(
                                    out=pt.bitcast(i16)[:, :, :il],
                                    in0=st[:, :, :il],
                                    scalar1=SCH_A,
                                    scalar2=SCH_B,
                                    op0=mult,
                                    op1=add,
                                )
                        else:
                            nc.scalar.activation(pt[:, :, :il], st[:, :, :il], Exp)
                        pts[j] = pt
                        if j >= 2:
                            emit_pv(j - 2, pts.pop(j - 2))
                    for j in (NJ - 2, NJ - 1):
                        emit_pv(j, pts.pop(j))

                    # Project the i-block from TWO chunks ago: its normalize
                    # chain (DMA roundtrips) finished long ago, so even if the
                    # Tile scheduler hoists these matmuls ahead of this
                    # chunk's PV stream, they never block the PE queue.
                    if p == 1 and nci >= 2:
                        emit_proj(*NCHUNKS[nci - 2])

                    # Normalize rows 0-63 by row 64 (softmax denominator).
                    # Copy both PV accumulators to SBUF immediately (frees the
                    # PSUM slots), then: both heads' denominator rows ->
                    # DRAM -> back as [128, W/128] so the exact reciprocal
                    # runs 128-wide on the DVE (a [1,512] reciprocal costs
                    # 3.6us; this costs ~0.1us) -> DRAM -> partition-broadcast
                    # read; final multiply on the otherwise idle GpSimd.
                    nsbs = []
                    for pv, h in ((pvA, hA), (pvB, hB)):
                        nsb = mp.tile([65, 512], f32, tag="nsb", name="nsb", bufs=4)
                        nc.vector.tensor_copy(nsb[:, :il], pv[:, :il])
                        nsbs.append((nsb, h))

                    pdst = [
                        outT[p][0:64, ns : ns + il],
                        outT[p][64:128, ns : ns + il],
                    ]
                    if p == 1 and nci == len(NCHUNKS) - 1:
                        # Tail short-path: the final chunk's normalize gates
                        # the last projection, so skip the DMA roundtrips:
                        # exact DVE reciprocal of the (short) denominator row,
                        # then broadcast across partitions with a K=1 ones
                        # matmul on the PE, multiply straight from PSUM.
                        with nc.allow_low_precision(
                            reason="attn out written bf16"
                        ):
                            for hi, (nsb, h) in enumerate(nsbs):
                                rden = mp.tile(
                                    [1, 512], f32, tag="rden", name="rden", bufs=2
                                )
                                nc.vector.reciprocal(
                                    rden[:, :il], nsb[64:65, :il]
                                )
                                bc_ps = ps_st.tile(
                                    [128, 2, 512], f32, tag="st", name="bc_ps"
                                )
                                nc.tensor.matmul(
                                    bc_ps[0:64, 0, :il],
                                    lhsT=ones64[:],
                                    rhs=rden[:, :il],
                                )
                                nc.vector.tensor_tensor(
                                    pdst[hi],
                                    nsb[0:64, :il],
                                    bc_ps[0:64, 0, :il],
                                    mult,
                                )
                        continue

                    W = 2 * il
                    rd = dp.tile([1, 1024], f32, tag="rd", name="rd")
                    rr = dp.tile([1, 1024], f32, tag="rr", name="rr")
                    for idx, (nsb, h) in enumerate(nsbs):
                        nc.sync.dma_start(
                            out=rd[:, idx * il : (idx + 1) * il],
                            in_=nsb[64:65, :il],
                        )
                    at = mp.tile([128, 8], f32, tag="at", name="at", bufs=4)
                    rc = mp.tile([128, 8], f32, tag="rc", name="rc", bufs=4)
                    nc.sync.dma_start(
                        out=at[:, : W // 128],
                        in_=rd[:, :W].rearrange("o (p c) -> (o p) c", p=128),
                    )
                    nc.vector.reciprocal(rc[:, : W // 128], at[:, : W // 128])
                    nc.sync.dma_start(
                        out=rr[:, :W].rearrange("o (p c) -> (o p) c", p=128),
                        in_=rc[:, : W // 128],
                    )
                    with nc.allow_low_precision(reason="attn out written bf16"):
                        for idx, (nsb, h) in enumerate(nsbs):
                            bc_sb = mp.tile(
                                [64, 512], f32, tag="bcsb", name="bc_sb", bufs=4
                            )
                            nc.sync.dma_start(
                                out=bc_sb[:, :il],
                                in_=rr[0:1, idx * il : idx * il + il].to_broadcast(
                                    (64, il)
                                ),
                            )
                            nc.gpsimd.tensor_tensor(
                                pdst[idx],
                                nsb[0:64, :il],
                                bc_sb[:, :il],
                                mult,
                            )

            emit_proj(*NCHUNKS[-2])
            emit_proj(*NCHUNKS[-1])

    return nc


_NC_CACHE = None


def kernel(x, w_qkv, w_out, b_out):
    global _NC_CACHE
    import ml_dtypes
    from concourse.bass_utils import run_bass_kernel_spmd

    bf = ml_dtypes.bfloat16
    x = np.ascontiguousarray(x, dtype=np.float32)
    w_qkv = np.asarray(w_qkv, dtype=np.float32)
    w_out = np.asarray(w_out, dtype=np.float32)
    b_out = np.asarray(b_out, dtype=np.float32)

    b, c, h, w = x.shape
    assert (b, c, h, w) == (B, C, 48, 48)
    x_bn = x.reshape(B, C, N)

    wq, wk, wv = w_qkv[0:HID], w_qkv[HID : 2 * HID], w_qkv[2 * HID : 3 * HID]
    w_outT = np.ascontiguousarray(w_out.T)  # [HID, C]

    in_maps = []
    for core in range(N_CORES):
        bb, g = core // 2, core % 2
        rows = slice(g * 256, g * 256 + 256)
        woutT_c = np.ascontiguousarray(
            w_outT[rows].reshape(2, 128, 256).transpose(1, 0, 2).astype(bf)
        )
        in_maps.append(
            {
                "x": np.ascontiguousarray(x_bn[bb].astype(bf)),
                "wqT": np.ascontiguousarray(wq[rows].T.astype(bf)),
                "wkT": np.ascontiguousarray(wk[rows].T.astype(bf)),
                "wvT": np.ascontiguousarray(wv[rows].T.astype(bf)),
                "woutT": woutT_c,
            }
        )

    if _NC_CACHE is None:
        _NC_CACHE = build_kernel()
    nc = _NC_CACHE

    trace = bool(int(os.environ.get("KERNEL_TRACE", "0")))
    res = run_bass_kernel_spmd(
        nc,
        in_maps,
        core_ids=list(range(N_CORES)),
        trace=trace,
        trace_cores=list(range(N_CORES)) if trace else None,
    )
    kernel.last_result = res

    y = np.empty((B, C, N), dtype=np.float32)
    for bb in range(B):
        y[bb] = (
            res.results[2 * bb]["y"]
            + res.results[2 * bb + 1]["y"]
            + b_out[:, None]
        )
    return y.reshape(B, C, 48, 48)


# revision 18
# speedup vs baseline: 1.4119x; 1.0289x over previous
"""Trainium2 Bass kernel for nn_Attention_16028817948779.

Reference computation (b=4, c=256, heads=8, d=64, h=w=48, n=2304):
  qkv = w_qkv @ x          (1x1 conv)
  q,k,v -> [b, H, d, n];  q,k l2-normalized along n (spatial)
  sim  = (q^T k) * 10;  attn = softmax(sim, axis=-1)
  out  = attn @ v^T -> [b, H, n, d] -> [b, H*d, h, w]
  y    = w_out @ out + b_out

Sharding: 8 cores; core c handles batch c//2, head group (c%2)*4..+4.
Each core computes a partial y over its 4 heads; host sums the two
partials per batch and adds the bias.

Kernel design (v2, bf16):
  - All matmul operands in bf16 (1 PE cycle/row vs ~1.5-2 for fp32r as
    measured on HW); inputs are cast to bf16 on the host so the DMA
    volume halves too. PSUM accumulation stays fp32; overall rel err
    ~5e-3 vs the 2e-2 gate.
  - Attention in transposed form ST[j,i] = k_j . q_i, so softmax needs no
    on-chip transposes: exp without max-subtraction is safe because q,k
    are l2-normalized along n (|sim| <= 10) and the softmax denominator
    comes for free from a ones-column appended to V^T in the PV matmul.
  - Two heads' ST matmuls run concurrently on the PE via row tiling
    (K=64 each; the head pair lives at partitions 0-63 / 64-127 of the
    QKV projection output).
  - 10/(|q_row| |k_row|) folded into a single per-row scale of q.
  - Softmax denominator: reciprocal_approx_fast straight from PSUM row
    64, partition-broadcast on the (otherwise idle) GpSimd engine, and
    one DVE multiply straight from PSUM -> bf16 outT. No DRAM roundtrip.
"""

import os
import sys

import numpy as np

_TRN_REPO = "/opt/trn_rl_repo"
if _TRN_REPO not in sys.path:
    sys.path.insert(0, _TRN_REPO)

B = 4
C = 256
HEADS = 8
D = 64
N = 2304  # 48*48
HID = HEADS * D  # 512

N_CORES = 8
CI = 2  # c chunks of 128
# i/n chunks of <=512 (PSUM bank / fp32 moving-operand limit)
NCHUNKS = [(0, 512), (512, 512), (1024, 512), (1536, 512), (2048, 256)]
NJ = N // 128  # 18 key chunks of 128


def _apply_compat_patches():
    """walrus in this env only accepts ~1 sync wait per instruction, but the
    Tile framework attaches one wait per outstanding proc to a single
    instruction. Split excess waits onto EventSemaphore instructions at the
    BIR-JSON level (Bass.to_json_bytes is the serialization choke point for
    both the native and the axon/PJRT compile paths)."""
    import json

    import concourse.bass as bass

    if getattr(bass.Bass.to_json_bytes, "_waitsplit", False):
        return

    MAXW = 1
    _orig = bass.Bass.to_json_bytes

    def _split_waits(raw):
        m = json.loads(raw)
        ctr = 0
        changed = False
        for f in m.get("functions", []):
            for blk in f.get("blocks", []):
                new_insts = []
                for ins in blk.get("instructions", []):
                    si = ins.get("sync_info")
                    waits = (si or {}).get("on_wait") or []
                    if len(waits) > MAXW:
                        changed = True
                        for w in waits[:-MAXW]:
                            ctr += 1
                            new_insts.append(
                                {
                                    "debug": ins.get("debug", 0),
                                    "engine": ins["engine"],
                                    "ins": [],
                                    "outs": [],
                                    "name": f"waitsplit_{ctr}",
                                    "opcode": "EventSemaphore",
                                    "sync_info": {"on_update": [], "on_wait": [w]},
                                }
                            )
                        si["on_wait"] = waits[-MAXW:]
                    new_insts.append(ins)
                blk["instructions"] = new_insts
        return json.dumps(m).encode() if changed else raw

    def _patched(self):
        return _split_waits(_orig(self))

    _patched._waitsplit = True
    bass.Bass.to_json_bytes = _patched


def build_kernel():
    import concourse.bass as bass
    import concourse.mybir as mybir
    import concourse.tile as tile

    _apply_compat_patches()

    f32 = mybir.dt.float32
    bf16 = mybir.dt.bfloat16
    i16 = mybir.dt.int16
    Exp = mybir.ActivationFunctionType.Exp
    Ln = mybir.ActivationFunctionType.Ln
    Square = mybir.ActivationFunctionType.Square
    mult = mybir.AluOpType.mult
    add = mybir.AluOpType.add
    X = mybir.AxisListType.X

    # Schraudolph bf16 exp: bits(int16(A*x + B)) read as bf16 ~= e^x
    # (max rel err 3.3%, verified bit-exact vs HW: DVE rounds-to-nearest).
    # Every third j-block's softmax exp runs on the DVE this way, relieving
    # the ACT engine (the kernel's bottleneck); softmax tolerates the error
    # (end-to-end rel err ~1.4e-2 vs the 2e-2 gate).
    SCH_A = float(128 * np.log2(np.e))
    SCH_B = float(127 * 128 - 5.5)

    nc = bass.Bass()
    x_d = nc.dram_tensor("x", [C, N], bf16, kind="ExternalInput")
    wqT_d = nc.dram_tensor("wqT", [C, 256], bf16, kind="ExternalInput")
    wkT_d = nc.dram_tensor("wkT", [C, 256], bf16, kind="ExternalInput")
    wvT_d = nc.dram_tensor("wvT", [C, 256], bf16, kind="ExternalInput")
    woutT_d = nc.dram_tensor("woutT", [128, 2, 256], bf16, kind="ExternalInput")
    y_d = nc.dram_tensor("y", [C, N], f32, kind="ExternalOutput")

    with tile.TileContext(nc) as tc:
        with (
            tc.tile_pool(name="persist", bufs=1) as pp,
            tc.tile_pool(name="pt", bufs=6) as ptp,
            tc.tile_pool(name="misc", bufs=2) as mp,
            tc.tile_pool(name="dram", bufs=2, space="DRAM") as dp,
            tc.tile_pool(name="ps_st", bufs=2, space="PSUM") as ps_st,
            tc.tile_pool(name="ps_pv", bufs=4, space="PSUM") as ps_pv,
        ):
            # ---- load inputs ----
            x_sb = pp.tile([128, CI, N], bf16)
            for ns, nl in NCHUNKS:
                for ci in range(CI):
                    nc.sync.dma_start(
                        out=x_sb[:, ci, ns : ns + nl],
                        in_=x_d[ci * 128 : (ci + 1) * 128, ns : ns + nl],
                    )
            wq_sb = pp.tile([128, CI, 256], bf16)
            wk_sb = pp.tile([128, CI, 256], bf16)
            wv_sb = pp.tile([128, CI, 256], bf16)
            for w_sb, w_d in ((wv_sb, wvT_d), (wq_sb, wqT_d), (wk_sb, wkT_d)):
                nc.sync.dma_start(
                    out=w_sb[:], in_=w_d.rearrange("(ci p) o -> p ci o", p=128)
                )
            wo_sb = pp.tile([128, 2, 256], bf16)
            nc.sync.dma_start(out=wo_sb[:], in_=woutT_d[:])

            ones_f = pp.tile([128, 1], bf16)
            nc.vector.memset(ones_f[:], 1.0)
            ones64 = pp.tile([1, 64], f32)
            nc.vector.memset(ones64[:], 1.0)

            # PE warm-up: dummy bf16 matmuls with no input dependencies,
            # executed during the initial DMA wait so the HAM clock gate is
            # already at 8/8 when the real QKV matmuls arrive.
            warm_sb = pp.tile([128, 512], bf16)
            nc.vector.memset(warm_sb[:], 1.0)
            warm_ps = ps_st.tile([128, 2, 512], f32, tag="st", name="warm_ps")
            NWARM = 8
            for wi in range(NWARM):
                nc.tensor.matmul(
                    warm_ps[:, 0, :],
                    lhsT=warm_sb[:, 0:128],
                    rhs=warm_sb[:],
                    start=(wi == 0),
                    stop=(wi == NWARM - 1),
                )
            nc.vector.tensor_copy(warm_sb[:, 0:16], warm_ps[:, 0, 0:16])

            # vt_sb: [n-part, j-chunk, 4*65]; per head 64 v columns + a ones col
            vt_sb = pp.tile([128, NJ, 260], bf16)
            vt4 = vt_sb.rearrange("p j (h e) -> p j h e", e=65)
            nc.vector.tensor_copy(
                vt4[:, :, :, 64:65],
                ones_f[:, 0:1].unsqueeze(1).unsqueeze(1).to_broadcast((128, NJ, 4, 1)),
            )
            for j in range(NJ):
                ps = ps_pv.tile([128, 256], f32, tag="pv", name="vt_ps")
                for ci in range(CI):
                    nc.tensor.matmul(
                        ps[:],
                        lhsT=x_sb[:, ci, j * 128 : (j + 1) * 128],
                        rhs=wv_sb[:, ci, :],
                        start=(ci == 0),
                        stop=(ci == CI - 1),
                    )
                eng = nc.scalar.copy if j % 2 == 0 else nc.vector.tensor_copy
                eng(vt4[:, j, :, 0:64], ps.rearrange("p (h d) -> p h d", h=4))

            # ---- QKV projection ----
            # q_sb/k_sb: [d-part, head-pair, n]; heads 2p at part 0-63, 2p+1 at 64-127
            q_sb = pp.tile([128, 2, N], bf16)
            k_sb = pp.tile([128, 2, N], bf16)
            # per-(tensor, oc) partial sum-of-squares, one column per n-chunk,
            # computed chunk-wise right behind the projection so the PE never
            # waits on the l2norm reduction later
            ssq = mp.tile([128, 2, 2, len(NCHUNKS)], f32, tag="ssq")
            scratch = pp.tile([128, 512], f32)
            for ti, (dst, w_sb) in enumerate(((q_sb, wq_sb), (k_sb, wk_sb))):
                for oc in range(2):
                    for nci, (ns, nl) in enumerate(NCHUNKS):
                        ps = ps_pv.tile([128, 512], f32, tag="pv", name="qk_ps")
                        for ci in range(CI):
                            nc.tensor.matmul(
                                ps[:, :nl],
                                lhsT=w_sb[:, ci, oc * 128 : (oc + 1) * 128],
                                rhs=x_sb[:, ci, ns : ns + nl],
                                start=(ci == 0),
                                stop=(ci == CI - 1),
                            )
                        nc.scalar.copy(dst[:, oc, ns : ns + nl], ps[:, :nl])
                        nc.vector.scalar_tensor_tensor(
                            scratch[:, :nl],
                            dst[:, oc, ns : ns + nl],
                            1.0,
                            dst[:, oc, ns : ns + nl],
                            mult,
                            mult,
                            accum_out=ssq[:, ti, oc, nci : nci + 1],
                        )

            # ---- fold l2norm + SCALE into q: q *= 10/sqrt(ssq_q*ssq_k) per row ----
            sqk = mp.tile([128, 2, 2], f32, tag="sqk")
            nc.vector.reduce_sum(
                sqk.rearrange("p a b -> p (a b)"),
                ssq.rearrange("p a b c -> p (a b) c"),
                axis=X,
            )
            qscale = mp.tile([128, 2], f32, tag="qscale")
            nc.vector.tensor_tensor(qscale[:], sqk[:, 0, :], sqk[:, 1, :], mult)
            # 10/sqrt(x) = exp(-0.5*ln(x) + ln(10)); Ln and Exp share one ACT
            # table set, so no extra table load next to the softmax exps
            nc.scalar.activation(qscale[:], qscale[:], Ln)
            ln10 = mp.tile([128, 1], f32, tag="ln10")
            nc.vector.memset(ln10[:], 2.302585092994046)
            nc.scalar.activation(
                qscale[:], qscale[:], Exp, bias=ln10[:], scale=-0.5
            )
            with nc.allow_low_precision(reason="q scale written as bf16"):
                # chunk-split so the first ST matmuls only wait on chunk 0
                for ns, nl in NCHUNKS:
                    for oc in range(2):
                        nc.vector.tensor_scalar_mul(
                            q_sb[:, oc, ns : ns + nl],
                            q_sb[:, oc, ns : ns + nl],
                            qscale[:, oc : oc + 1],
                        )

            # ---- attention per head pair p (local heads 2p, 2p+1) ----
            # outT[p] stacks the pair's two heads: head 2p at partitions
            # 0-63, head 2p+1 at 64-127 (engines support partition-shifted
            # writes), so the projection contracts K=128 per pair.
            outT = [
                pp.tile([128, N], bf16, name=f"outT{pp_}", tag=f"outT{pp_}")
                for pp_ in range(2)
            ]

            def emit_proj(ns, il):
                for oc_ in range(2):
                    yps = ps_pv.tile([128, 512], f32, tag="pv", name="yps")
                    for pr in range(2):
                        nc.tensor.matmul(
                            yps[:, :il],
                            lhsT=wo_sb[:, pr, oc_ * 128 : (oc_ + 1) * 128],
                            rhs=outT[pr][:, ns : ns + il],
                            start=(pr == 0),
                            stop=(pr == 1),
                        )
                    y_sb = mp.tile([128, 512], f32, tag="ysb", name="y_sb")
                    nc.vector.tensor_copy(y_sb[:, :il], yps[:, :il])
                    nc.sync.dma_start(
                        out=y_d[oc_ * 128 : (oc_ + 1) * 128, ns : ns + il],
                        in_=y_sb[:, :il],
                    )

            for p in range(2):
                for nci, (ns, il) in enumerate(NCHUNKS):
                    hA, hB = 2 * p, 2 * p + 1
                    pvA = ps_pv.tile([65, 512], f32, tag="pv", name="pvA")
                    pvB = ps_pv.tile([65, 512], f32, tag="pv", name="pvB")

                    def emit_pv(jj, pt):
                        nc.tensor.matmul(
                            pvA[:, :il],
                            lhsT=vt4[:, jj, hA, :],
                            rhs=pt[:, 0, :il],
                            start=(jj == 0),
                            stop=(jj == NJ - 1),
                        )
                        nc.tensor.matmul(
                            pvB[:, :il],
                            lhsT=vt4[:, jj, hB, :],
                            rhs=pt[:, 1, :il],
                            start=(jj == 0),
                            stop=(jj == NJ - 1),
                        )

                    # Software-pipeline the PV matmuls two steps behind the
                    # ST/exp stream: the PE queue is strictly in-order, so
                    # with PV(j) emitted right after ST(j) the queue stalls
                    # on exp(j) (~1.2us latency) every step. With lag 2 the
                    # exp hides behind two steps of PE work.
                    pts = {}
                    for j in range(NJ):
                        st = ps_st.tile([128, 2, 512], f32, tag="st", name="st")
                        nc.tensor.matmul(
                            st[:, 0, :il],
                            lhsT=k_sb[0:64, p, j * 128 : (j + 1) * 128],
                            rhs=q_sb[0:64, p, ns : ns + il],
                        )
                        nc.tensor.matmul(
                            st[:, 1, :il],
                            lhsT=k_sb[64:128, p, j * 128 : (j + 1) * 128],
                            rhs=q_sb[64:128, p, ns : ns + il],
                        )
                        pt = ptp.tile([128, 2, 512], bf16, tag="pt", name="pt")
                        if (j % 2 == 1) if il == 256 else (j % 4 == 3):
                            # Schraudolph exp on the DVE (ACT is the
                            # bottleneck engine; this takes every 3rd j)
                            with nc.allow_low_precision(
                                reason="approx exp bits written as int16"
                            ):
                                nc.vector.tensor_scalar(
                                    out=pt.bitcast(i16)[:, :, :il],
                                    in0=st[:, :, :il],
                                    scalar1=SCH_A,
                                    scalar2=SCH_B,
                                    op0=mult,
                                    op1=add,
                                )
                        else:
                            nc.scalar.activation(pt[:, :, :il], st[:, :, :il], Exp)
                        pts[j] = pt
                        if j >= 2:
                            emit_pv(j - 2, pts.pop(j - 2))
                    for j in (NJ - 2, NJ - 1):
                        emit_pv(j, pts.pop(j))

                    # Project the i-block from TWO chunks ago: its normalize
                    # chain (DMA roundtrips) finished long ago, so even if the
                    # Tile scheduler hoists these matmuls ahead of this
                    # chunk's PV stream, they never block the PE queue.
                    if p == 1 and nci >= 2:
                        emit_proj(*NCHUNKS[nci - 2])

                    # Normalize rows 0-63 by row 64 (softmax denominator).
                    # Copy both PV accumulators to SBUF immediately (frees the
                    # PSUM slots), then: both heads' denominator rows ->
                    # DRAM -> back as [128, W/128] so the exact reciprocal
                    # runs 128-wide on the DVE (a [1,512] reciprocal costs
                    # 3.6us; this costs ~0.1us) -> DRAM -> partition-broadcast
                    # read; final multiply on the otherwise idle GpSimd.
                    nsbs = []
                    for pv, h in ((pvA, hA), (pvB, hB)):
                        nsb = mp.tile([65, 512], f32, tag="nsb", name="nsb", bufs=4)
                        nc.vector.tensor_copy(nsb[:, :il], pv[:, :il])
                        nsbs.append((nsb, h))

                    pdst = [
                        outT[p][0:64, ns : ns + il],
                        outT[p][64:128, ns : ns + il],
                    ]
                    if p == 1 and nci == len(NCHUNKS) - 1:
                        # Tail short-path: the final chunk's normalize gates
                        # the last projection, so skip the DMA roundtrips:
                        # exact DVE reciprocal of the (short) denominator row,
                        # then broadcast across partitions with a K=1 ones
                        # matmul on the PE, multiply straight from PSUM.
                        with nc.allow_low_precision(
                            reason="attn out written bf16"
                        ):
                            for hi, (nsb, h) in enumerate(nsbs):
                                rden = mp.tile(
                                    [1, 512], f32, tag="rden", name="rden", bufs=2
                                )
                                nc.vector.reciprocal(
                                    rden[:, :il], nsb[64:65, :il]
                                )
                                bc_ps = ps_st.tile(
                                    [128, 2, 512], f32, tag="st", name="bc_ps"
                                )
                                nc.tensor.matmul(
                                    bc_ps[0:64, 0, :il],
                                    lhsT=ones64[:],
                                    rhs=rden[:, :il],
                                )
                                nc.vector.tensor_tensor(
                                    pdst[hi],
                                    nsb[0:64, :il],
                                    bc_ps[0:64, 0, :il],
                                    mult,
                                )
                        continue

                    W = 2 * il
                    rd = dp.tile([1, 1024], f32, tag="rd", name="rd")
                    rr = dp.tile([1, 1024], f32, tag="rr", name="rr")
                    for idx, (nsb, h) in enumerate(nsbs):
                        nc.sync.dma_start(
                            out=rd[:, idx * il : (idx + 1) * il],
                            in_=nsb[64:65, :il],
                        )
                    at = mp.tile([128, 8], f32, tag="at", name="at", bufs=4)
                    rc = mp.tile([128, 8], f32, tag="rc", name="rc", bufs=4)
                    nc.sync.dma_start(
                        out=at[:, : W // 128],
                        in_=rd[:, :W].rearrange("o (p c) -> (o p) c", p=128),
                    )
                    nc.vector.reciprocal(rc[:, : W // 128], at[:, : W // 128])
                    nc.sync.dma_start(
                        out=rr[:, :W].rearrange("o (p c) -> (o p) c", p=128),
                        in_=rc[:, : W // 128],
                    )
                    with nc.allow_low_precision(reason="attn out written bf16"):
                        for idx, (nsb, h) in enumerate(nsbs):
                            bc_sb = mp.tile(
                                [64, 512], f32, tag="bcsb", name="bc_sb", bufs=4
                            )
                            nc.sync.dma_start(
                                out=bc_sb[:, :il],
                                in_=rr[0:1, idx * il : idx * il + il].to_broadcast(
                                    (64, il)
                                ),
                            )
                            nc.gpsimd.tensor_tensor(
                                pdst[idx],
                                nsb[0:64, :il],
                                bc_sb[:, :il],
                                mult,
                            )

            emit_proj(*NCHUNKS[-2])
            emit_proj(*NCHUNKS[-1])

    return nc


_NC_CACHE = None


def kernel(x, w_qkv, w_out, b_out):
    global _NC_CACHE
    import ml_dtypes
    from concourse.bass_utils import run_bass_kernel_spmd

    bf = ml_dtypes.bfloat16
    x = np.ascontiguousarray(x, dtype=np.float32)
    w_qkv = np.asarray(w_qkv, dtype=np.float32)
    w_out = np.asarray(w_out, dtype=np.float32)
    b_out = np.asarray(b_out, dtype=np.float32)

    b, c, h, w = x.shape
    assert (b, c, h, w) == (B, C, 48, 48)
    x_bn = x.reshape(B, C, N)

    wq, wk, wv = w_qkv[0:HID], w_qkv[HID : 2 * HID], w_qkv[2 * HID : 3 * HID]
    w_outT = np.ascontiguousarray(w_out.T)  # [HID, C]

    in_maps = []
    for core in range(N_CORES):
        bb, g = core // 2, core % 2
        rows = slice(g * 256, g * 256 + 256)
        woutT_c = np.ascontiguousarray(
            w_outT[rows].reshape(2, 128, 256).transpose(1, 0, 2).astype(bf)
        )
        in_maps.append(
            {
                "x": np.ascontiguousarray(x_bn[bb].astype(bf)),
                "wqT": np.ascontiguousarray(wq[rows].T.astype(bf)),
                "wkT": np.ascontiguousarray(wk[rows].T.astype(bf)),
                "wvT": np.ascontiguousarray(wv[rows].T.astype(bf)),
                "woutT": woutT_c,
            }
        )

    if _NC_CACHE is None:
        _NC_CACHE = build_kernel()
    nc = _NC_CACHE

    trace = bool(int(os.environ.get("KERNEL_TRACE", "0")))
    res = run_bass_kernel_spmd(
        nc,
        in_maps,
        core_ids=list(range(N_CORES)),
        trace=trace,
        trace_cores=list(range(N_CORES)) if trace else None,
    )
    kernel.last_result = res

    y = np.empty((B, C, N), dtype=np.float32)
    for bb in range(B):
        y[bb] = (
            res.results[2 * bb]["y"]
            + res.results[2 * bb + 1]["y"]
            + b_out[:, None]
        )
    return y.reshape(B, C, 48, 48)
